# revision 1
# baseline (speedup 1.0000x reference)
"""Trainium2 Bass kernel for nn_AttentionLayerBase (relative-position banded attention).

Sharding: 16 heads over 8 cores (2 heads/core, tensor-parallel). Each core:
  - Q^T,K^T (f-major) and V (t-major) + per-head R^T projections from X^T
  - P_rev = R @ reversed(b_nd) per q-tile, written to DRAM, re-read with a
    diagonal-stride DMA to materialize extra[i,j] = P[i, i-j] (Toeplitz gather)
  - S = Q K^T/sqrt(d) + extra (bias added via identity-matmul PSUM accumulate)
  - exp on ACT with fused row-sums; causal mask pre-applied on the bias tile
  - A^T tiles via PE transpose with diag(1/rowsum) as the moving operand
    (fuses softmax normalization into the transpose)
  - out^T = V^T-style accumulation, then partial output projection
Host sums the 8 partial projections and adds b_proj.
"""

import numpy as np

import concourse.bass as bass
import concourse.mybir as mybir
import concourse.tile as tile
from concourse import bacc
from concourse.masks import make_identity

FP = mybir.dt.float32
N_HEADS = 16
N_CORES = 8
HPC = N_HEADS // N_CORES  # heads per core = 2
E = 1024
DH = 64            # head dim (qk and v)
FQ = HPC * DH      # per-core q/k/v feature cols = 128
NB = 10            # n_basis
FR = HPC * NB      # per-core r cols = 20
ML = 2048          # max_len of b_nd
SCALE = 1.0 / 8.0  # 1/sqrt(64)


def build_nc(T):
    NT = T // 128
    nc = bacc.Bacc("TRN2", target_bir_lowering=False, debug=False)

    xt_d = nc.dram_tensor("xt", [E, T], FP, kind="ExternalInput")
    wq_d = nc.dram_tensor("wq", [E, FQ], FP, kind="ExternalInput")
    wk_d = nc.dram_tensor("wk", [E, FQ], FP, kind="ExternalInput")
    wv_d = nc.dram_tensor("wv", [E, FQ], FP, kind="ExternalInput")
    wr_d = nc.dram_tensor("wr", [E, FR], FP, kind="ExternalInput")
    wp_d = nc.dram_tensor("wp", [HPC, DH, E], FP, kind="ExternalInput")
    bq_d = nc.dram_tensor("bq8", [FQ, 1], FP, kind="ExternalInput")
    br_d = nc.dram_tensor("br", [NB, HPC], FP, kind="ExternalInput")
    brev_d = nc.dram_tensor("brev", [NB, ML], FP, kind="ExternalInput")
    out_d = nc.dram_tensor("out_partial", [T, E], FP, kind="ExternalOutput")

    # DRAM scratch for the sheared bias: one [128, ML] block per (head, q-tile)
    pdram = [
        nc.dram_tensor(f"prev{h}", [NT, 128, ML], FP, kind="Internal")
        for h in range(HPC)
    ]

    with tile.TileContext(nc) as tc:
        with (
            tc.tile_pool(name="const", bufs=1) as const_pool,
            tc.tile_pool(name="big", bufs=1) as big_pool,
            tc.tile_pool(name="work", bufs=2) as work_pool,
            tc.tile_pool(name="small", bufs=3) as small_pool,
            tc.tile_pool(name="ps_s", bufs=2, space="PSUM") as ps_s,
            tc.tile_pool(name="ps_p", bufs=2, space="PSUM") as ps_p,
            tc.tile_pool(name="ps_t", bufs=2, space="PSUM") as ps_t,
            tc.tile_pool(name="ps_av", bufs=1, space="PSUM") as ps_av,
            tc.tile_pool(name="ps_o", bufs=1, space="PSUM") as ps_o,
        ):
            # ---- constants / weights into SBUF ----
            ident = const_pool.tile([128, 128], FP)
            make_identity(nc, ident[:])

            xt_sb = big_pool.tile([128, E // 128, T], FP)
            nc.sync.dma_start(xt_sb[:], xt_d.rearrange("(po pi) t -> pi po t", pi=128))
            wq_sb = const_pool.tile([128, E // 128, FQ], FP)
            nc.sync.dma_start(wq_sb[:], wq_d.rearrange("(po pi) f -> pi po f", pi=128))
            wk_sb = const_pool.tile([128, E // 128, FQ], FP)
            nc.sync.dma_start(wk_sb[:], wk_d.rearrange("(po pi) f -> pi po f", pi=128))
            wv_sb = const_pool.tile([128, E // 128, FQ], FP)
            nc.sync.dma_start(wv_sb[:], wv_d.rearrange("(po pi) f -> pi po f", pi=128))
            wr_sb = const_pool.tile([128, E // 128, FR], FP)
            nc.sync.dma_start(wr_sb[:], wr_d.rearrange("(po pi) f -> pi po f", pi=128))
            wp_sb = const_pool.tile([DH, HPC, E], FP)
            nc.sync.dma_start(wp_sb[:], wp_d.rearrange("h p f -> p h f"))
            bq_sb = const_pool.tile([FQ, 1], FP)
            nc.sync.dma_start(bq_sb[:], bq_d[:])
            br_sb = const_pool.tile([NB, HPC], FP)
            nc.sync.dma_start(br_sb[:], br_d[:])
            brev_sb = const_pool.tile([NB, ML], FP)
            nc.sync.dma_start(brev_sb[:], brev_d[:])

            # ---- projections ----
            qT_sb = big_pool.tile([FQ, T], FP)   # rows: head-major qk features
            kT_sb = big_pool.tile([FQ, T], FP)
            rT_sb = [big_pool.tile([NB, T], FP, name=f"rT{h}") for h in range(HPC)]
            v_sb = big_pool.tile([128, NT, FQ], FP)   # V t-major
            outT_sb = [big_pool.tile([DH, T], FP, name=f"oT{h}") for h in range(HPC)]

            NE = E // 128
            for c0 in range(0, T, 512):
                ct = min(c0 + 512, T)
                # Q^T chunk
                qp = ps_s.tile([128, 512], FP, name="qp", tag="mm")
                for e in range(NE):
                    nc.tensor.matmul(qp[:FQ, :ct - c0], wq_sb[:, e], xt_sb[:, e, c0:ct],
                                     start=(e == 0), stop=(e == NE - 1))
                nc.scalar.activation(qT_sb[:, c0:ct], qp[:FQ, :ct - c0],
                                     mybir.ActivationFunctionType.Identity,
                                     bias=bq_sb[:, 0:1], scale=SCALE)
                # K^T chunk
                kp = ps_s.tile([128, 512], FP, name="kp", tag="mm")
                for e in range(NE):
                    nc.tensor.matmul(kp[:FQ, :ct - c0], wk_sb[:, e], xt_sb[:, e, c0:ct],
                                     start=(e == 0), stop=(e == NE - 1))
                nc.any.tensor_copy(kT_sb[:, c0:ct], kp[:FQ, :ct - c0])
                # R^T per head
                for h in range(HPC):
                    rp = ps_p.tile([128, 512], FP, name="rp", tag="pp")[:NB]
                    for e in range(NE):
                        nc.tensor.matmul(rp[:, :ct - c0], wr_sb[:, e, NB * h:NB * (h + 1)],
                                         xt_sb[:, e, c0:ct],
                                         start=(e == 0), stop=(e == NE - 1))
                    nc.scalar.activation(rT_sb[h][:, c0:ct], rp[:, :ct - c0],
                                         mybir.ActivationFunctionType.Identity,
                                         bias=br_sb[:, h:h + 1], scale=1.0)
            for ti in range(NT):
                vp = ps_s.tile([128, 512], FP, name="vp", tag="mm")[:, :FQ]
                for e in range(NE):
                    nc.tensor.matmul(vp[:], xt_sb[:, e, ti * 128:(ti + 1) * 128],
                                     wv_sb[:, e], start=(e == 0), stop=(e == NE - 1))
                nc.any.tensor_copy(v_sb[:, ti], vp[:])

            # ---- attention per head ----
            for h in range(HPC):
                qh = qT_sb[DH * h:DH * (h + 1)]
                kh = kT_sb[DH * h:DH * (h + 1)]
                for ti in range(NT):
                    W = 128 * (ti + 1)
                    # P_rev band for this q-tile: columns [ML-W, ML)
                    for cs in range(ML - W, ML, 512):
                        ce = min(cs + 512, ML)
                        pp = ps_p.tile([128, 512], FP, name="pp", tag="pp")
                        nc.tensor.matmul(pp[:, :ce - cs],
                                         rT_sb[h][:, ti * 128:(ti + 1) * 128],
                                         brev_sb[:, cs:ce], start=True, stop=True)
                        pband = small_pool.tile([128, 512], FP, name="pband")
                        nc.any.tensor_copy(pband[:, :ce - cs], pp[:, :ce - cs])
                        nc.sync.dma_start(pdram[h][ti, :, cs:ce], pband[:, :ce - cs])
                    # diagonal gather: extra[p, j] = P_rev[128ti+p, ML-1-128ti-p+j]
                    extra = work_pool.tile([128, T], FP, name="extra")
                    src = bass.AP(pdram[h], ti * 128 * ML + (ML - 1 - 128 * ti),
                                  [[ML - 1, 128], [1, W]])
                    nc.sync.dma_start(extra[:, :W], src)
                    # causal mask on the diagonal 128-block (also kills junk reads)
                    nc.gpsimd.affine_select(
                        out=extra[:, W - 128:W], in_=extra[:, W - 128:W],
                        compare_op=mybir.AluOpType.is_ge, fill=-1e9,
                        base=0, channel_multiplier=1, pattern=[[-1, 128]])

                    # S = Q K^T/8 + extra, then exp with fused row sums
                    expb = work_pool.tile([128, T], FP, name="expb")
                    sums4 = small_pool.tile([128, T // 512 + 1], FP, name="sums4")
                    nch = 0
                    for cs in range(0, W, 512):
                        ce = min(cs + 512, W)
                        sp = ps_s.tile([128, 512], FP, name="sp", tag="mm")
                        nc.tensor.matmul(sp[:, :ce - cs], qh[:, ti * 128:(ti + 1) * 128],
                                         kh[:, cs:ce], start=True, stop=False)
                        nc.tensor.matmul(sp[:, :ce - cs], ident[:], extra[:, cs:ce],
                                         start=False, stop=True)
                        nc.scalar.activation(expb[:, cs:ce], sp[:, :ce - cs],
                                             mybir.ActivationFunctionType.Exp,
                                             accum_out=sums4[:, nch:nch + 1])
                        nch += 1
                    sums1 = small_pool.tile([128, 1], FP, name="sums1")
                    if nch > 1:
                        nc.vector.tensor_reduce(sums1[:], sums4[:, :nch],
                                                axis=mybir.AxisListType.X,
                                                op=mybir.AluOpType.add)
                    else:
                        nc.vector.tensor_copy(sums1[:], sums4[:, 0:1])
                    recip = small_pool.tile([128, 1], FP, name="recip")
                    nc.vector.reciprocal(recip[:], sums1[:])
                    nc.vector.tensor_scalar_mul(expb[:, :W], expb[:, :W], recip[:, 0:1])

                    # A^T tiles + AV accumulate
                    avp = ps_av.tile([DH, 128], FP, name="avp")
                    for j in range(ti + 1):
                        tp = ps_t.tile([128, 128], FP, name="tp")
                        nc.tensor.matmul(tp[:], expb[:, j * 128:(j + 1) * 128],
                                         ident[:], is_transpose=True)
                        aT = small_pool.tile([128, 128], FP, name="aT")
                        nc.any.tensor_copy(aT[:], tp[:])
                        nc.tensor.matmul(avp[:], v_sb[:, j, DH * h:DH * (h + 1)],
                                         aT[:], start=(j == 0), stop=(j == ti))
                    nc.any.tensor_copy(outT_sb[h][:, ti * 128:(ti + 1) * 128], avp[:])

            # ---- partial output projection ----
            for ti in range(NT):
                po_sb = work_pool.tile([128, E], FP, name="po")
                for n0 in range(0, E, 512):
                    op = ps_o.tile([128, 512], FP, name="op")
                    for h in range(HPC):
                        nc.tensor.matmul(op[:], outT_sb[h][:, ti * 128:(ti + 1) * 128],
                                         wp_sb[:, h, n0:n0 + 512],
                                         start=(h == 0), stop=(h == HPC - 1))
                    nc.any.tensor_copy(po_sb[:, n0:n0 + 512], op[:])
                nc.sync.dma_start(out_d[ti * 128:(ti + 1) * 128, :], po_sb[:])

    nc.compile()
    return nc


def make_in_maps(inputs, T):
    X = np.asarray(inputs["X_bte"], np.float32)[0]  # (T, E)
    xt = np.ascontiguousarray(X.T)                  # (E, T)
    brev = np.ascontiguousarray(np.asarray(inputs["b_nd"], np.float32)[:, ::-1])
    in_maps = []
    for c in range(N_CORES):
        fq = slice(FQ * c, FQ * (c + 1))
        fr = slice(FR * c, FR * (c + 1))
        wr_c = np.ascontiguousarray(np.asarray(inputs["W_r"], np.float32)[:, fr])
        br_c = np.asarray(inputs["b_r"], np.float32)[fr].reshape(HPC, NB).T  # [NB, HPC]
        wp_c = np.asarray(inputs["W_proj"], np.float32)[fq, :].reshape(HPC, DH, -1)
        in_maps.append({
            "xt": xt,
            "wq": np.ascontiguousarray(np.asarray(inputs["W_q"], np.float32)[:, fq]),
            "wk": np.ascontiguousarray(np.asarray(inputs["W_k"], np.float32)[:, fq]),
            "wv": np.ascontiguousarray(np.asarray(inputs["W_v"], np.float32)[:, fq]),
            "wr": wr_c,
            "wp": np.ascontiguousarray(wp_c),
            "bq8": (np.asarray(inputs["b_q"], np.float32)[fq] * SCALE).reshape(FQ, 1),
            "br": np.ascontiguousarray(br_c),
            "brev": brev,
        })
    return in_maps


_NC_CACHE = {}
LAST_RESULTS = None


def kernel(**inputs):
    T = np.asarray(inputs["X_bte"]).shape[1]
    if T not in _NC_CACHE:
        _NC_CACHE[T] = build_nc(T)
    nc = _NC_CACHE[T]
    in_maps = make_in_maps(inputs, T)
    from concourse.bass_utils import run_bass_kernel_spmd
    res = run_bass_kernel_spmd(nc, in_maps, core_ids=list(range(N_CORES)))
    global LAST_RESULTS
    LAST_RESULTS = res
    acc = np.zeros((T, E), np.float64)
    for r in res.results:
        acc += r["out_partial"].astype(np.float64)
    acc += np.asarray(inputs["b_proj"], np.float32)[None, :]
    return acc.astype(np.float32)[None]



# revision 3
# speedup vs baseline: 2.1409x; 2.1409x over previous
"""Trainium2 Bass kernel for nn_AttentionLayerBase (relative-position banded attention).

Sharding: 16 heads over 8 cores (2 heads/core, tensor parallel). All matmuls in
bf16 (inputs host-cast), f32 PSUM accumulation. Per core:
  - Q^T,K^T (feature-major, heads at partition 0-63 / 64-127), V (t-major) and
    R^T (heads at partitions 0-9 / 32-41 for PE row-group packing) from X^T
  - per q-tile: P_rev band = R @ reversed(b_nd) (both heads' matmuls packed in
    disjoint PE row groups), evicted to SBUF in bf16, then a diagonal-stride
    SBUF->SBUF DMA materializes extra[p,j] = P[p, p-j] (Toeplitz shear, no HBM)
  - S = Q K^T/sqrt(d) (heads packed in row groups 0-1/2-3) + extra via DVE add
    into PSUM; exp on ACT with fused row-sum accumulation (output bf16)
  - softmax normalization on DVE (per-partition reciprocal scale, in place)
  - A^T tiles via PE transpose; out^T accumulated with V-stationary matmuls
  - output projection: single 128-deep contraction over both heads' context
Partial projections written as bf16; host sums the 8 partials + b_proj in f32.
"""

import numpy as np
import ml_dtypes

import concourse.bass as bass
import concourse.mybir as mybir
import concourse.tile as tile
from concourse import bacc
from concourse.masks import make_identity

FP = mybir.dt.float32
BF = mybir.dt.bfloat16
BF_NP = ml_dtypes.bfloat16
N_HEADS = 16
N_CORES = 8
HPC = N_HEADS // N_CORES  # heads per core = 2
E = 1024
DH = 64            # head dim (qk and v)
FQ = HPC * DH      # per-core q/k/v feature cols = 128
NB = 10            # n_basis
FR = HPC * NB      # per-core r cols = 20
ML = 2048          # max_len of b_nd
SCALE = 1.0 / 8.0  # 1/sqrt(64)
NEG = -1e9


def build_nc(T):
    NT = T // 128
    NE = E // 128
    nc = bacc.Bacc("TRN2", target_bir_lowering=False, debug=False)

    xt_d = nc.dram_tensor("xt", [E, T], BF, kind="ExternalInput")
    wq_d = nc.dram_tensor("wq", [E, FQ], BF, kind="ExternalInput")
    wk_d = nc.dram_tensor("wk", [E, FQ], BF, kind="ExternalInput")
    wv_d = nc.dram_tensor("wv", [E, FQ], BF, kind="ExternalInput")
    wr_d = nc.dram_tensor("wr", [E, FR], BF, kind="ExternalInput")
    wp_d = nc.dram_tensor("wp", [FQ, E], BF, kind="ExternalInput")
    bq_d = nc.dram_tensor("bq8", [FQ, 1], FP, kind="ExternalInput")
    br_d = nc.dram_tensor("br", [42, 1], FP, kind="ExternalInput")
    brev_d = nc.dram_tensor("brev", [42, ML], BF, kind="ExternalInput")
    out_d = nc.dram_tensor("out_partial", [T, E], BF, kind="ExternalOutput")

    with tile.TileContext(nc) as tc:
        with (
            tc.tile_pool(name="const", bufs=1) as const_pool,
            tc.tile_pool(name="big", bufs=1) as big_pool,
            tc.tile_pool(name="work", bufs=2) as work_pool,
            tc.tile_pool(name="small", bufs=3) as small_pool,
            tc.tile_pool(name="ps_s", bufs=2, space="PSUM") as ps_s,
            tc.tile_pool(name="ps_p", bufs=2, space="PSUM") as ps_p,
            tc.tile_pool(name="ps_t", bufs=2, space="PSUM") as ps_t,
            tc.tile_pool(name="ps_av", bufs=2, space="PSUM") as ps_av,
        ):
            # ---- constants / weights into SBUF ----
            ident = const_pool.tile([128, 128], BF)
            make_identity(nc, ident[:])

            xt_sb = big_pool.tile([128, NE, T], BF)
            nc.sync.dma_start(xt_sb[:], xt_d.rearrange("(po pi) t -> pi po t", pi=128))
            wq_sb = const_pool.tile([128, NE, FQ], BF)
            nc.sync.dma_start(wq_sb[:], wq_d.rearrange("(po pi) f -> pi po f", pi=128))
            wk_sb = const_pool.tile([128, NE, FQ], BF)
            nc.sync.dma_start(wk_sb[:], wk_d.rearrange("(po pi) f -> pi po f", pi=128))
            wv_sb = const_pool.tile([128, NE, FQ], BF)
            nc.sync.dma_start(wv_sb[:], wv_d.rearrange("(po pi) f -> pi po f", pi=128))
            wr_sb = const_pool.tile([128, NE, FR], BF)
            nc.sync.dma_start(wr_sb[:], wr_d.rearrange("(po pi) f -> pi po f", pi=128))
            wp_sb = const_pool.tile([128, E], BF)
            nc.sync.dma_start(wp_sb[:], wp_d[:])
            bq_sb = const_pool.tile([FQ, 1], FP)
            nc.sync.dma_start(bq_sb[:], bq_d[:])
            br_sb = const_pool.tile([42, 1], FP)
            nc.sync.dma_start(br_sb[:], br_d[:])
            brev_sb = const_pool.tile([42, ML], BF)
            nc.sync.dma_start(brev_sb[:], brev_d[:])

            # ---- projections ----
            qT_sb = big_pool.tile([128, T], BF)   # head h at partitions 64h:64h+64
            kT_sb = big_pool.tile([128, T], BF)
            rT_sb = big_pool.tile([42, T], BF)    # head h at partitions 32h:32h+10
            v_sb = big_pool.tile([128, NT, FQ], BF)    # t-major; head h cols 64h:64h+64
            outcT_sb = big_pool.tile([128, T], BF)     # (h,d)-major context^T

            for c0 in range(0, T, 512):
                ct = min(c0 + 512, T)
                qp = ps_s.tile([128, 512], FP, name="qp", tag="mm")
                for e in range(NE):
                    nc.tensor.matmul(qp[:, :ct - c0], wq_sb[:, e], xt_sb[:, e, c0:ct],
                                     start=(e == 0), stop=(e == NE - 1))
                nc.scalar.activation(qT_sb[:, c0:ct], qp[:, :ct - c0],
                                     mybir.ActivationFunctionType.Identity,
                                     bias=bq_sb[:, 0:1], scale=SCALE)
                kp = ps_s.tile([128, 512], FP, name="kp", tag="mm")
                for e in range(NE):
                    nc.tensor.matmul(kp[:, :ct - c0], wk_sb[:, e], xt_sb[:, e, c0:ct],
                                     start=(e == 0), stop=(e == NE - 1))
                nc.vector.tensor_copy(kT_sb[:, c0:ct], kp[:, :ct - c0])
                # R^T both heads in one psum tile, col groups 0 / 1
                rp = ps_p.tile([128, 512], FP, name="rp", tag="pp")
                for e in range(NE):
                    nc.tensor.matmul(rp[0:NB, :ct - c0], wr_sb[:, e, 0:NB],
                                     xt_sb[:, e, c0:ct],
                                     start=(e == 0), stop=(e == NE - 1))
                    nc.tensor.matmul(rp[32:32 + NB, :ct - c0], wr_sb[:, e, NB:2 * NB],
                                     xt_sb[:, e, c0:ct],
                                     start=(e == 0), stop=(e == NE - 1))
                nc.scalar.activation(rT_sb[0:NB, c0:ct], rp[0:NB, :ct - c0],
                                     mybir.ActivationFunctionType.Identity,
                                     bias=br_sb[0:NB, 0:1], scale=1.0)
                nc.scalar.activation(rT_sb[32:32 + NB, c0:ct], rp[32:32 + NB, :ct - c0],
                                     mybir.ActivationFunctionType.Identity,
                                     bias=br_sb[32:32 + NB, 0:1], scale=1.0)
            for ti in range(NT):
                vp = ps_s.tile([128, 512], FP, name="vp", tag="mm")[:, :FQ]
                for e in range(NE):
                    nc.tensor.matmul(vp[:], xt_sb[:, e, ti * 128:(ti + 1) * 128],
                                     wv_sb[:, e], start=(e == 0), stop=(e == NE - 1))
                nc.vector.tensor_copy(v_sb[:, ti], vp[:])

            # ---- attention ----
            for ti in range(NT):
                W = 128 * (ti + 1)
                CW = W + 128
                # P_rev bands for both heads (row-group packed matmuls)
                pbands = [work_pool.tile([128, T + 128], BF, name=f"pband{h}")
                          for h in range(HPC)]
                for cs in range(0, W, 512):
                    ce = min(cs + 512, W)
                    pps = []
                    for h in range(HPC):
                        pp = ps_p.tile([128, 512], FP, name="pp", tag="pp")
                        nc.tensor.matmul(pp[:, :ce - cs],
                                         rT_sb[32 * h:32 * h + NB, ti * 128:(ti + 1) * 128],
                                         brev_sb[32 * h:32 * h + NB, ML - W + cs:ML - W + ce],
                                         start=True, stop=True)
                        pps.append(pp)
                    for h in range(HPC):
                        nc.vector.tensor_copy(pbands[h][:, cs:ce], pps[h][:, :ce - cs])
                # Toeplitz shear: extra[p, j] = pband[p, 127 - p + j] (SBUF->SBUF DMA)
                extras = []
                for h in range(HPC):
                    extra = work_pool.tile([128, T], BF, name=f"extra{h}")
                    # stride is relative to the tile's flat row length (T+128)
                    src = bass.AP(pbands[h][:].tensor, 127, [[T + 128 - 1, 128], [1, W]])
                    nc.sync.dma_start(extra[:, :W], src)
                    # causal mask on diagonal block (also kills junk shear reads)
                    nc.gpsimd.affine_select(
                        out=extra[:, W - 128:W], in_=extra[:, W - 128:W],
                        compare_op=mybir.AluOpType.is_ge, fill=NEG,
                        base=0, channel_multiplier=1, pattern=[[-1, 128]])
                    extras.append(extra)

                # S = Q K^T/8 + extra, exp with fused row sums
                aexps = [work_pool.tile([128, T], BF, name=f"aexp{h}")
                         for h in range(HPC)]
                sums4 = [small_pool.tile([128, T // 512 + 1], FP, name=f"sums4{h}")
                         for h in range(HPC)]
                nch = 0
                for cs in range(0, W, 512):
                    ce = min(cs + 512, W)
                    sps = []
                    for h in range(HPC):
                        sp = ps_s.tile([128, 512], FP, name="sp", tag="mm")
                        nc.tensor.matmul(sp[:, :ce - cs],
                                         qT_sb[64 * h:64 * h + DH, ti * 128:(ti + 1) * 128],
                                         kT_sb[64 * h:64 * h + DH, cs:ce],
                                         start=True, stop=True)
                        sps.append(sp)
                    for h in range(HPC):
                        nc.vector.tensor_tensor(sps[h][:, :ce - cs], sps[h][:, :ce - cs],
                                                extras[h][:, cs:ce], mybir.AluOpType.add)
                        nc.scalar.activation(aexps[h][:, cs:ce], sps[h][:, :ce - cs],
                                             mybir.ActivationFunctionType.Exp,
                                             accum_out=sums4[h][:, nch:nch + 1])
                    nch += 1

                for h in range(HPC):
                    sums1 = small_pool.tile([128, 1], FP, name="sums1")
                    if nch > 1:
                        nc.vector.tensor_reduce(sums1[:], sums4[h][:, :nch],
                                                axis=mybir.AxisListType.X,
                                                op=mybir.AluOpType.add)
                    else:
                        nc.vector.tensor_copy(sums1[:], sums4[h][:, 0:1])
                    recip = small_pool.tile([128, 1], FP, name="recip")
                    nc.vector.reciprocal(recip[:], sums1[:])
                    nc.vector.tensor_scalar_mul(aexps[h][:, :W], aexps[h][:, :W],
                                                recip[:, 0:1])

                    # A^T tiles + AV accumulate (out^T form, V stationary)
                    avp = ps_av.tile([DH, 128], FP, name="avp")
                    for j in range(ti + 1):
                        tp = ps_t.tile([128, 128], BF, name="tp")
                        nc.tensor.matmul(tp[:], aexps[h][:, j * 128:(j + 1) * 128],
                                         ident[:], is_transpose=True)
                        aT = small_pool.tile([128, 128], BF, name="aT")
                        nc.vector.tensor_copy(aT[:], tp[:])
                        nc.tensor.matmul(avp[:], v_sb[:, j, DH * h:DH * (h + 1)],
                                         aT[:], start=(j == 0), stop=(j == ti))
                    nc.vector.tensor_copy(
                        outcT_sb[64 * h:64 * h + DH, ti * 128:(ti + 1) * 128], avp[:])

                # ---- output projection for this q-tile ----
                po_sb = work_pool.tile([128, E], BF, name="po")
                for n0 in range(0, E, 512):
                    op = ps_s.tile([128, 512], FP, name="op", tag="mm")
                    nc.tensor.matmul(op[:], outcT_sb[:, ti * 128:(ti + 1) * 128],
                                     wp_sb[:, n0:n0 + 512], start=True, stop=True)
                    nc.scalar.copy(po_sb[:, n0:n0 + 512], op[:])
                nc.sync.dma_start(out_d[ti * 128:(ti + 1) * 128, :], po_sb[:])

    nc.compile()
    return nc


def make_in_maps(inputs, T):
    X = np.asarray(inputs["X_bte"], np.float32)[0]  # (T, E)
    xt = np.ascontiguousarray(X.T).astype(BF_NP)    # (E, T)
    brev_1 = np.asarray(inputs["b_nd"], np.float32)[:, ::-1].astype(BF_NP)
    brev = np.zeros((42, ML), BF_NP)
    brev[0:NB] = brev_1
    brev[32:32 + NB] = brev_1
    W_q = np.asarray(inputs["W_q"], np.float32)
    W_k = np.asarray(inputs["W_k"], np.float32)
    W_v = np.asarray(inputs["W_v"], np.float32)
    W_r = np.asarray(inputs["W_r"], np.float32)
    W_p = np.asarray(inputs["W_proj"], np.float32)
    b_q = np.asarray(inputs["b_q"], np.float32)
    b_r = np.asarray(inputs["b_r"], np.float32)
    in_maps = []
    for c in range(N_CORES):
        fq = slice(FQ * c, FQ * (c + 1))
        fr = slice(FR * c, FR * (c + 1))
        br = np.zeros((42, 1), np.float32)
        br[0:NB, 0] = b_r[FR * c:FR * c + NB]
        br[32:32 + NB, 0] = b_r[FR * c + NB:FR * c + 2 * NB]
        in_maps.append({
            "xt": xt,
            "wq": np.ascontiguousarray(W_q[:, fq]).astype(BF_NP),
            "wk": np.ascontiguousarray(W_k[:, fq]).astype(BF_NP),
            "wv": np.ascontiguousarray(W_v[:, fq]).astype(BF_NP),
            "wr": np.ascontiguousarray(W_r[:, fr]).astype(BF_NP),
            "wp": np.ascontiguousarray(W_p[fq, :]).astype(BF_NP),
            "bq8": (b_q[fq] * SCALE).reshape(FQ, 1).astype(np.float32),
            "br": br,
            "brev": brev,
        })
    return in_maps


_NC_CACHE = {}
LAST_RESULTS = None


def kernel(**inputs):
    T = np.asarray(inputs["X_bte"]).shape[1]
    if T not in _NC_CACHE:
        _NC_CACHE[T] = build_nc(T)
    nc = _NC_CACHE[T]
    in_maps = make_in_maps(inputs, T)
    from concourse.bass_utils import run_bass_kernel_spmd
    res = run_bass_kernel_spmd(nc, in_maps, core_ids=list(range(N_CORES)))
    global LAST_RESULTS
    LAST_RESULTS = res
    acc = np.zeros((T, E), np.float32)
    for r in res.results:
        acc += r["out_partial"].astype(np.float32)
    acc += np.asarray(inputs["b_proj"], np.float32)[None, :]
    return acc.astype(np.float32)[None]


# revision 5
# speedup vs baseline: 2.8660x; 1.3387x over previous
"""Trainium2 Bass kernel for nn_AttentionLayerBase (relative-position banded attention).

Sharding: 16 heads over 8 cores (2 heads/core, tensor parallel). All matmuls in
bf16 (inputs host-cast), f32 PSUM accumulation. Per core:
  - Q^T,K^T (feature-major, heads at partition 0-63 / 64-127), V (t-major) and
    R^T (heads at partitions 0-9 / 32-41 for PE row-group packing) from X^T
  - per q-tile: P_rev band = R @ reversed(b_nd) (both heads' matmuls packed in
    disjoint PE row groups), evicted to SBUF in bf16, then a diagonal-stride
    SBUF->SBUF DMA materializes extra[p,j] = P[p, p-j] (Toeplitz shear, no HBM)
  - S = Q K^T/sqrt(d) (heads packed in row groups 0-1/2-3) + extra via DVE add
    into PSUM; exp on ACT with fused row-sum accumulation (output bf16)
  - softmax normalization on DVE (per-partition reciprocal scale, in place)
  - A^T tiles via PE transpose; out^T accumulated with V-stationary matmuls
  - output projection: single 128-deep contraction over both heads' context
Partial projections written as bf16; host sums the 8 partials + b_proj in f32.
"""

import numpy as np
import ml_dtypes

import concourse.bass as bass
import concourse.mybir as mybir
import concourse.tile as tile
from concourse import bacc
from concourse.masks import make_identity

FP = mybir.dt.float32
BF = mybir.dt.bfloat16
BF_NP = ml_dtypes.bfloat16
N_HEADS = 16
N_CORES = 8
HPC = N_HEADS // N_CORES  # heads per core = 2
E = 1024
DH = 64            # head dim (qk and v)
FQ = HPC * DH      # per-core q/k/v feature cols = 128
NB = 10            # n_basis
FR = HPC * NB      # per-core r cols = 20
ML = 2048          # max_len of b_nd
SCALE = 1.0 / 8.0  # 1/sqrt(64)
NEG = -1e9


def build_nc(T):
    NT = T // 128
    NE = E // 128
    nc = bacc.Bacc("TRN2", target_bir_lowering=False, debug=False)

    xt_d = nc.dram_tensor("xt", [E, T], BF, kind="ExternalInput")
    wq_d = nc.dram_tensor("wq", [E, FQ], BF, kind="ExternalInput")
    wk_d = nc.dram_tensor("wk", [E, FQ], BF, kind="ExternalInput")
    wv_d = nc.dram_tensor("wv", [E, FQ], BF, kind="ExternalInput")
    wr_d = nc.dram_tensor("wr", [E, FR], BF, kind="ExternalInput")
    wp_d = nc.dram_tensor("wp", [FQ, E], BF, kind="ExternalInput")
    bq_d = nc.dram_tensor("bq8", [FQ, 1], FP, kind="ExternalInput")
    br_d = nc.dram_tensor("br", [42, 1], FP, kind="ExternalInput")
    brev_d = nc.dram_tensor("brev", [42, ML], BF, kind="ExternalInput")
    out_d = nc.dram_tensor("out_partial", [T, E], BF, kind="ExternalOutput")

    with tile.TileContext(nc) as tc:
        with (
            tc.tile_pool(name="const", bufs=1) as const_pool,
            tc.tile_pool(name="big", bufs=1) as big_pool,
            tc.tile_pool(name="work", bufs=2) as work_pool,
            tc.tile_pool(name="small", bufs=3) as small_pool,
            tc.tile_pool(name="ps_s", bufs=2, space="PSUM") as ps_s,
            tc.tile_pool(name="ps_p", bufs=2, space="PSUM") as ps_p,
            tc.tile_pool(name="ps_t", bufs=2, space="PSUM") as ps_t,
            tc.tile_pool(name="ps_av", bufs=2, space="PSUM") as ps_av,  # [128,128] f32 = 1 bank each
        ):
            # ---- constants / weights into SBUF ----
            ident = const_pool.tile([128, 128], BF)
            make_identity(nc, ident[:])

            xt_sb = big_pool.tile([128, NE, T], BF)
            nc.sync.dma_start(xt_sb[:], xt_d.rearrange("(po pi) t -> pi po t", pi=128))
            wq_sb = const_pool.tile([128, NE, FQ], BF)
            nc.sync.dma_start(wq_sb[:], wq_d.rearrange("(po pi) f -> pi po f", pi=128))
            wk_sb = const_pool.tile([128, NE, FQ], BF)
            nc.sync.dma_start(wk_sb[:], wk_d.rearrange("(po pi) f -> pi po f", pi=128))
            wv_sb = const_pool.tile([128, NE, FQ], BF)
            nc.sync.dma_start(wv_sb[:], wv_d.rearrange("(po pi) f -> pi po f", pi=128))
            wr_sb = const_pool.tile([128, NE, FR], BF)
            nc.sync.dma_start(wr_sb[:], wr_d.rearrange("(po pi) f -> pi po f", pi=128))
            wp_sb = const_pool.tile([128, E], BF)
            nc.sync.dma_start(wp_sb[:], wp_d[:])
            bq_sb = const_pool.tile([FQ, 1], FP)
            nc.sync.dma_start(bq_sb[:], bq_d[:])
            br_sb = const_pool.tile([42, 1], FP)
            nc.sync.dma_start(br_sb[:], br_d[:])
            brev_sb = const_pool.tile([42, ML], BF)
            nc.sync.dma_start(brev_sb[:], brev_d[:])

            # ---- projections ----
            qT_sb = big_pool.tile([128, T], BF)   # head h at partitions 64h:64h+64
            kT_sb = big_pool.tile([128, T], BF)
            rT_sb = big_pool.tile([42, T], BF)    # head h at partitions 32h:32h+10
            v_sb = big_pool.tile([128, NT, FQ], BF)    # t-major; head h cols 64h:64h+64
            outcT_sb = big_pool.tile([128, T], BF)     # (h,d)-major context^T

            for c0 in range(0, T, 512):
                ct = min(c0 + 512, T)
                qp = ps_s.tile([128, 512], FP, name="qp", tag="mm")
                for e in range(NE):
                    nc.tensor.matmul(qp[:, :ct - c0], wq_sb[:, e], xt_sb[:, e, c0:ct],
                                     start=(e == 0), stop=(e == NE - 1))
                nc.scalar.activation(qT_sb[:, c0:ct], qp[:, :ct - c0],
                                     mybir.ActivationFunctionType.Identity,
                                     bias=bq_sb[:, 0:1], scale=SCALE)
                kp = ps_s.tile([128, 512], FP, name="kp", tag="mm")
                for e in range(NE):
                    nc.tensor.matmul(kp[:, :ct - c0], wk_sb[:, e], xt_sb[:, e, c0:ct],
                                     start=(e == 0), stop=(e == NE - 1))
                nc.vector.tensor_copy(kT_sb[:, c0:ct], kp[:, :ct - c0])
                # R^T both heads in one psum tile, col groups 0 / 1
                rp = ps_p.tile([128, 512], FP, name="rp", tag="pp")
                for e in range(NE):
                    nc.tensor.matmul(rp[0:NB, :ct - c0], wr_sb[:, e, 0:NB],
                                     xt_sb[:, e, c0:ct],
                                     start=(e == 0), stop=(e == NE - 1))
                    nc.tensor.matmul(rp[32:32 + NB, :ct - c0], wr_sb[:, e, NB:2 * NB],
                                     xt_sb[:, e, c0:ct],
                                     start=(e == 0), stop=(e == NE - 1))
                nc.scalar.activation(rT_sb[0:NB, c0:ct], rp[0:NB, :ct - c0],
                                     mybir.ActivationFunctionType.Identity,
                                     bias=br_sb[0:NB, 0:1], scale=1.0)
                nc.scalar.activation(rT_sb[32:32 + NB, c0:ct], rp[32:32 + NB, :ct - c0],
                                     mybir.ActivationFunctionType.Identity,
                                     bias=br_sb[32:32 + NB, 0:1], scale=1.0)
            for ti in range(NT):
                vp = ps_s.tile([128, 512], FP, name="vp", tag="mm")[:, :FQ]
                for e in range(NE):
                    nc.tensor.matmul(vp[:], xt_sb[:, e, ti * 128:(ti + 1) * 128],
                                     wv_sb[:, e], start=(e == 0), stop=(e == NE - 1))
                nc.vector.tensor_copy(v_sb[:, ti], vp[:])

            # ---- attention ----
            for ti in range(NT):
                W = 128 * (ti + 1)
                CW = W + 128
                # P_rev bands for both heads (row-group packed matmuls)
                pbands = [work_pool.tile([128, T + 128], BF, name=f"pband{h}")
                          for h in range(HPC)]
                for cs in range(0, W, 512):
                    ce = min(cs + 512, W)
                    pps = []
                    for h in range(HPC):
                        pp = ps_p.tile([128, 512], FP, name="pp", tag="pp")
                        nc.tensor.matmul(pp[:, :ce - cs],
                                         rT_sb[32 * h:32 * h + NB, ti * 128:(ti + 1) * 128],
                                         brev_sb[32 * h:32 * h + NB, ML - W + cs:ML - W + ce],
                                         start=True, stop=True)
                        pps.append(pp)
                    for h in range(HPC):
                        nc.vector.tensor_copy(pbands[h][:, cs:ce], pps[h][:, :ce - cs])
                # Toeplitz shear: extra[p, j] = pband[p, 127 - p + j] (SBUF->SBUF DMA)
                extras = []
                for h in range(HPC):
                    extra = work_pool.tile([128, T], BF, name=f"extra{h}")
                    # stride is relative to the tile's flat row length (T+128)
                    src = bass.AP(pbands[h][:].tensor, 127, [[T + 128 - 1, 128], [1, W]])
                    nc.sync.dma_start(extra[:, :W], src)
                    # causal mask on diagonal block (also kills junk shear reads)
                    nc.gpsimd.affine_select(
                        out=extra[:, W - 128:W], in_=extra[:, W - 128:W],
                        compare_op=mybir.AluOpType.is_ge, fill=NEG,
                        base=0, channel_multiplier=1, pattern=[[-1, 128]])
                    extras.append(extra)

                # S = Q K^T/8 + extra, exp with fused row sums
                aexps = [work_pool.tile([128, T], BF, name=f"aexp{h}")
                         for h in range(HPC)]
                sums4 = [small_pool.tile([128, T // 512 + 1], FP, name=f"sums4{h}")
                         for h in range(HPC)]
                nch = 0
                for cs in range(0, W, 512):
                    ce = min(cs + 512, W)
                    sps = []
                    for h in range(HPC):
                        sp = ps_s.tile([128, 512], FP, name="sp", tag="mm")
                        nc.tensor.matmul(sp[:, :ce - cs],
                                         qT_sb[64 * h:64 * h + DH, ti * 128:(ti + 1) * 128],
                                         kT_sb[64 * h:64 * h + DH, cs:ce],
                                         start=True, stop=True)
                        sps.append(sp)
                    for h in range(HPC):
                        nc.vector.tensor_tensor(sps[h][:, :ce - cs], sps[h][:, :ce - cs],
                                                extras[h][:, cs:ce], mybir.AluOpType.add)
                        nc.scalar.activation(aexps[h][:, cs:ce], sps[h][:, :ce - cs],
                                             mybir.ActivationFunctionType.Exp,
                                             accum_out=sums4[h][:, nch:nch + 1])
                    nch += 1

                # row sums -> reciprocals (applied later at out-proj eviction,
                # off the PE critical path; A stays unnormalized)
                recips = []
                for h in range(HPC):
                    sums1 = small_pool.tile([128, 1], FP, name="sums1")
                    if nch > 1:
                        nc.vector.tensor_reduce(sums1[:], sums4[h][:, :nch],
                                                axis=mybir.AxisListType.X,
                                                op=mybir.AluOpType.add)
                    else:
                        nc.vector.tensor_copy(sums1[:], sums4[h][:, 0:1])
                    recip = small_pool.tile([128, 1], FP, name=f"recip{h}")
                    nc.vector.reciprocal(recip[:], sums1[:])
                    recips.append(recip)

                # A^T tiles (4-batched transposes) + AV accumulate; both heads'
                # AV matmuls col-packed into one PSUM tile [128 (h,d), 128 q]
                avp = ps_av.tile([128, 128], FP, name="avp")
                for j0 in range(0, ti + 1, 4):
                    j1 = min(j0 + 4, ti + 1)
                    aT4s = []
                    for h in range(HPC):
                        tp4 = ps_t.tile([128, 512], BF, name="tp4")
                        for c in range(j1 - j0):
                            j = j0 + c
                            nc.tensor.matmul(tp4[:, c * 128:(c + 1) * 128],
                                             aexps[h][:, j * 128:(j + 1) * 128],
                                             ident[:], is_transpose=True)
                        aT4 = small_pool.tile([128, 512], BF, name="aT4")
                        nc.vector.tensor_copy(aT4[:, :(j1 - j0) * 128],
                                              tp4[:, :(j1 - j0) * 128])
                        aT4s.append(aT4)
                    for c in range(j1 - j0):
                        j = j0 + c
                        for h in range(HPC):
                            nc.tensor.matmul(avp[64 * h:64 * h + DH, :],
                                             v_sb[:, j, DH * h:DH * (h + 1)],
                                             aT4s[h][:, c * 128:(c + 1) * 128],
                                             start=(j == 0), stop=(j == ti))
                nc.vector.tensor_copy(outcT_sb[:, ti * 128:(ti + 1) * 128], avp[:])

                # ---- output projection for this q-tile (per head, row-packed;
                # softmax 1/rowsum applied at eviction) ----
                po_sb = work_pool.tile([128, E], BF, name="po")
                for n0 in range(0, E, 512):
                    ops = []
                    for h in range(HPC):
                        op = ps_s.tile([128, 512], FP, name=f"op{h}", tag="mm")
                        nc.tensor.matmul(op[:], outcT_sb[64 * h:64 * h + DH,
                                                         ti * 128:(ti + 1) * 128],
                                         wp_sb[64 * h:64 * h + DH, n0:n0 + 512],
                                         start=True, stop=True)
                        ops.append(op)
                    nc.scalar.mul(po_sb[:, n0:n0 + 512], ops[0][:], recips[0][:, 0:1])
                    nc.vector.scalar_tensor_tensor(
                        po_sb[:, n0:n0 + 512], ops[1][:], recips[1][:, 0:1],
                        po_sb[:, n0:n0 + 512],
                        mybir.AluOpType.mult, mybir.AluOpType.add)
                nc.sync.dma_start(out_d[ti * 128:(ti + 1) * 128, :], po_sb[:])

    nc.compile()
    return nc


def make_in_maps(inputs, T):
    X = np.asarray(inputs["X_bte"], np.float32)[0]  # (T, E)
    xt = np.ascontiguousarray(X.T).astype(BF_NP)    # (E, T)
    brev_1 = np.asarray(inputs["b_nd"], np.float32)[:, ::-1].astype(BF_NP)
    brev = np.zeros((42, ML), BF_NP)
    brev[0:NB] = brev_1
    brev[32:32 + NB] = brev_1
    W_q = np.asarray(inputs["W_q"], np.float32)
    W_k = np.asarray(inputs["W_k"], np.float32)
    W_v = np.asarray(inputs["W_v"], np.float32)
    W_r = np.asarray(inputs["W_r"], np.float32)
    W_p = np.asarray(inputs["W_proj"], np.float32)
    b_q = np.asarray(inputs["b_q"], np.float32)
    b_r = np.asarray(inputs["b_r"], np.float32)
    in_maps = []
    for c in range(N_CORES):
        fq = slice(FQ * c, FQ * (c + 1))
        fr = slice(FR * c, FR * (c + 1))
        br = np.zeros((42, 1), np.float32)
        br[0:NB, 0] = b_r[FR * c:FR * c + NB]
        br[32:32 + NB, 0] = b_r[FR * c + NB:FR * c + 2 * NB]
        in_maps.append({
            "xt": xt,
            "wq": np.ascontiguousarray(W_q[:, fq]).astype(BF_NP),
            "wk": np.ascontiguousarray(W_k[:, fq]).astype(BF_NP),
            "wv": np.ascontiguousarray(W_v[:, fq]).astype(BF_NP),
            "wr": np.ascontiguousarray(W_r[:, fr]).astype(BF_NP),
            "wp": np.ascontiguousarray(W_p[fq, :]).astype(BF_NP),
            "bq8": (b_q[fq] * SCALE).reshape(FQ, 1).astype(np.float32),
            "br": br,
            "brev": brev,
        })
    return in_maps


_NC_CACHE = {}
LAST_RESULTS = None


def kernel(**inputs):
    T = np.asarray(inputs["X_bte"]).shape[1]
    if T not in _NC_CACHE:
        _NC_CACHE[T] = build_nc(T)
    nc = _NC_CACHE[T]
    in_maps = make_in_maps(inputs, T)
    from concourse.bass_utils import run_bass_kernel_spmd
    res = run_bass_kernel_spmd(nc, in_maps, core_ids=list(range(N_CORES)))
    global LAST_RESULTS
    LAST_RESULTS = res
    acc = np.zeros((T, E), np.float32)
    for r in res.results:
        acc += r["out_partial"].astype(np.float32)
    acc += np.asarray(inputs["b_proj"], np.float32)[None, :]
    return acc.astype(np.float32)[None]
